# revision 16
# baseline (speedup 1.0000x reference)
"""YOLO-style detector decode kernel for Trainium2 (8 NeuronCores, SPMD).

Sharding: pure data parallel over the batch dim (128 -> 16 per core).

Per-core layout: for each head (13/26/52) the 45 channels are 3 anchors x 15
roles (iou, dx, dy, dw, dh, 10 class scores).  Each role is gathered into an
SBUF tile [P, F] where the partition dim enumerates (g, b, a) (g = half of the
cell space, to use 96 of 128 partitions) and the free dim enumerates cells.
Decode is then elementwise over role planes:

  pre   = grid*stride + stride*dx          (iota grid + ACT copy-scale + DVE add)
  half  = exp(dw + ln(anchor/2))           (single ACT op, bias folds the mul)
  x1/x2 = pre -/+ half
  mask  = iou > thresh
  kind  = tournament argmax over the 10 class planes

Outputs are written as dense planes ob[6, B, 3, HW] (+ mask [B, 3, HW]) per
head; the host interleaves them into the reference [N, 6] ordering (cheap
numpy transpose, keeps every DMA fully contiguous).
"""

import numpy as np

import concourse.bass as bass
import concourse.bacc as bacc
import concourse.mybir as mybir
from concourse.tile import TileContext
from concourse.bass_utils import run_bass_kernel_spmd

F32 = mybir.dt.float32
U8 = mybir.dt.uint8
ALU = mybir.AluOpType
ACTF = mybir.ActivationFunctionType

NCORES = 8
B = 128
BL = B // NCORES  # 16

# name, H(=W), stride, G (cell-space split to widen partitions), free chunk
HEADS = [
    ("13", 13, 32, 1, 169),
    ("26", 26, 16, 2, 338),
    ("52", 52, 8, 2, 676),
]


def _build_nc():
    # Bacc (not raw Bass): its compile() legalizes multi-wait sync_info into
    # event semaphores (hardware allows one wait per instruction).
    nc = bacc.Bacc("TRN2", target_bir_lowering=False)
    ins = {}
    obs = {}
    oms = {}
    for name, H, S, G, FC in HEADS:
        HW = H * H
        ins[name] = nc.declare_dram_parameter(f"in{name}", [BL, 45, HW], F32, isOutput=False)
        obs[name] = nc.declare_dram_parameter(f"ob{name}", [6, BL, 3, HW], F32, isOutput=True)
        oms[name] = nc.declare_dram_parameter(f"om{name}", [BL, 3, HW], U8, isOutput=True)
    consts = nc.declare_dram_parameter("consts", [96, 8], F32, isOutput=False)

    with TileContext(nc) as tc:
        with (
            tc.tile_pool(name="persist", bufs=1) as ppool,
            tc.tile_pool(name="work", bufs=2) as wpool,
            tc.tile_pool(name="scratch", bufs=1) as spool,
        ):
            consts_t = ppool.tile([96, 8], F32, tag="consts")
            nc.sync.dma_start(out=consts_t[:, :], in_=consts[:])

            # Per-head grid tiles (stride-scaled cell coordinates), built once
            # on gpsimd via iota.  Values <= 408, exact in f32.
            grids = {}
            for name, H, S, G, FC in HEADS:
                P = 48 * G
                H2 = H // G
                F = H2 * H
                gx = ppool.tile([P, F], F32, tag=f"gx{name}")
                gy = ppool.tile([P, F], F32, tag=f"gy{name}")
                nc.gpsimd.iota(
                    gx[:, :], pattern=[[0, H2], [S, H]], base=0,
                    channel_multiplier=0, allow_small_or_imprecise_dtypes=True,
                )
                # iota can't start at partition 48 (and partition windows must
                # be 32-aligned), so write the g=0 pattern everywhere; the
                # g=1 half-offset (H2*S = 208 for both split heads) is folded
                # into the sdy activation bias below via consts col 7.
                nc.gpsimd.iota(
                    gy[:, :], pattern=[[S, H2], [0, H]], base=0,
                    channel_multiplier=0, allow_small_or_imprecise_dtypes=True,
                )
                grids[name] = (gx, gy)

            for hi, (name, H, S, G, FC) in enumerate(HEADS):
                HW = H * H
                P = 48 * G
                F = HW // G
                gx, gy = grids[name]
                in_r = ins[name][:].rearrange("b (a r) (g f) -> g b a r f", a=3, g=G)
                ob_r = obs[name][:].rearrange("r b a (g f) -> r g b a f", g=G)
                om_r = oms[name][:].rearrange("b a (g f) -> g b a f", g=G)
                cw, ch = 2 * hi, 2 * hi + 1

                for cs in range(0, F, FC):
                    ce = cs + FC
                    t = []
                    for k in range(15):
                        tk = wpool.tile([P, FC], F32, tag=f"r{k}")
                        nc.sync.dma_start(out=tk[:, :], in_=in_r[:, :, :, k, cs:ce])
                        t.append(tk)

                    sdx = wpool.tile([P, FC], F32, tag="sdx")
                    sdy = wpool.tile([P, FC], F32, tag="sdy")
                    hw2 = wpool.tile([P, FC], F32, tag="hw2")
                    hh2 = wpool.tile([P, FC], F32, tag="hh2")
                    nc.scalar.activation(sdx[:, :], t[1][:, :], ACTF.Copy, scale=float(S))
                    # bias adds the g-half y offset (0 for g=0 / head13 rows)
                    nc.scalar.activation(sdy[:, :], t[2][:, :], ACTF.Identity,
                                         scale=float(S), bias=consts_t[0:P, 7:8])
                    nc.scalar.activation(hw2[:, :], t[3][:, :], ACTF.Exp, bias=consts_t[0:P, cw:cw + 1])
                    nc.scalar.activation(hh2[:, :], t[4][:, :], ACTF.Exp, bias=consts_t[0:P, ch:ch + 1])

                    # pre_x/pre_y in place on sdx/sdy
                    nc.vector.tensor_tensor(sdx[:, :], sdx[:, :], gx[:, cs:ce], ALU.add)
                    nc.vector.tensor_tensor(sdy[:, :], sdy[:, :], gy[:, cs:ce], ALU.add)
                    x1 = wpool.tile([P, FC], F32, tag="x1")
                    y1 = wpool.tile([P, FC], F32, tag="y1")
                    nc.vector.tensor_tensor(x1[:, :], sdx[:, :], hw2[:, :], ALU.subtract)
                    nc.vector.tensor_tensor(y1[:, :], sdy[:, :], hh2[:, :], ALU.subtract)
                    # x2/y2 in place on sdx/sdy
                    nc.vector.tensor_tensor(sdx[:, :], sdx[:, :], hw2[:, :], ALU.add)
                    nc.vector.tensor_tensor(sdy[:, :], sdy[:, :], hh2[:, :], ALU.add)

                    # mask = iou > thresh, without TensorScalarPtr (1 wait slot
                    # only): sign(iou - thresh) is 1 iff strictly greater,
                    # then relu maps {-1,0,1} -> {0,1}.  consts col 6 = -thresh.
                    msk = wpool.tile([P, FC], U8, tag="msk")
                    sgn = wpool.tile([P, FC], F32, tag="sgn")
                    nc.scalar.activation(sgn[:, :], t[0][:, :], ACTF.Sign, bias=consts_t[0:P, 6:7])
                    nc.scalar.activation(msk[:, :], sgn[:, :], ACTF.Relu)

                    # tournament argmax over class planes t[5..14]
                    c = t[5:]
                    m = []   # running max planes (in place on even class planes)
                    ix = []  # running argmax planes
                    for i in range(5):
                        gt = spool.tile([P, FC], U8, tag=f"gt{i}")
                        idx = spool.tile([P, FC], F32, tag=f"idx{i}")
                        nc.vector.tensor_tensor(gt[:, :], c[2 * i + 1][:, :], c[2 * i][:, :], ALU.is_gt)
                        nc.vector.tensor_tensor(c[2 * i][:, :], c[2 * i][:, :], c[2 * i + 1][:, :], ALU.max)
                        nc.vector.tensor_scalar(
                            out=idx[:, :], in0=gt[:, :],
                            scalar1=float(2 * i), scalar2=None, op0=ALU.add,
                        )
                        m.append(c[2 * i])
                        ix.append(idx)

                    def merge(i, j):
                        g2 = spool.tile([P, FC], U8, tag="gm")
                        nc.vector.tensor_tensor(g2[:, :], m[j][:, :], m[i][:, :], ALU.is_gt)
                        nc.vector.copy_predicated(ix[i][:, :], g2[:, :], ix[j][:, :])
                        nc.vector.tensor_tensor(m[i][:, :], m[i][:, :], m[j][:, :], ALU.max)

                    merge(0, 1)
                    merge(2, 3)
                    merge(0, 2)
                    merge(0, 4)

                    for ri, plane in ((0, t[0]), (1, x1), (2, y1), (3, sdx), (4, sdy), (5, ix[0])):
                        nc.sync.dma_start(out=ob_r[ri, :, :, :, cs:ce], in_=plane[:, :])
                    nc.sync.dma_start(out=om_r[:, :, :, cs:ce], in_=msk[:, :])
    nc.compile()
    return nc


_NC_CACHE = None


def _get_nc():
    global _NC_CACHE
    if _NC_CACHE is None:
        _NC_CACHE = _build_nc()
    return _NC_CACHE


def _host_inputs(output13, output26, output52, anchors13, anchors26, anchors52, thresh):
    consts = np.zeros((96, 8), np.float32)
    amod = np.arange(96) % 3
    for hi, anc in enumerate((anchors13, anchors26, anchors52)):
        anc = np.asarray(anc, np.float32)
        consts[:, 2 * hi] = np.log(anc[:, 0] / 2.0)[amod]
        consts[:, 2 * hi + 1] = np.log(anc[:, 1] / 2.0)[amod]
    consts[:, 6] = -np.float32(thresh)
    consts[:, 7] = np.where(np.arange(96) >= 48, 208.0, 0.0)
    full = {"13": np.asarray(output13, np.float32),
            "26": np.asarray(output26, np.float32),
            "52": np.asarray(output52, np.float32)}
    in_maps = []
    for cid in range(NCORES):
        m = {"consts": consts}
        for name in full:
            sl = np.ascontiguousarray(full[name][cid * BL:(cid + 1) * BL])
            m[f"in{name}"] = sl.reshape(BL, 45, -1)
        in_maps.append(m)
    return in_maps


def _assemble(results):
    boxes_parts = []
    mask_parts = []
    for name, H, S, G, FC in HEADS:
        ob = np.concatenate([r[f"ob{name}"] for r in results], axis=1)  # [6,128,3,HW]
        om = np.concatenate([r[f"om{name}"] for r in results], axis=0)  # [128,3,HW]
        boxes_parts.append(np.ascontiguousarray(ob.transpose(1, 3, 2, 0)).reshape(-1, 6))
        mask_parts.append((om.transpose(0, 2, 1) != 0).reshape(-1))
    return np.concatenate(boxes_parts, 0), np.concatenate(mask_parts, 0)


def _run(trace=False, **inputs):
    nc = _get_nc()
    in_maps = _host_inputs(**inputs)
    res = run_bass_kernel_spmd(nc, in_maps, list(range(NCORES)), trace=trace)
    out = _assemble(res.results)
    return out, res


def kernel(**inputs):
    out, _ = _run(trace=False, **inputs)
    return out


def kernel_traced(**inputs):
    return _run(trace=True, **inputs)


# revision 19
# speedup vs baseline: 24.7619x; 24.7619x over previous
"""YOLO-style detector decode kernel for Trainium2 (8 NeuronCores, SPMD).

Sharding: pure data parallel over the batch dim (128 -> 16 per core).

Per-core layout: for each head (13/26/52) the 45 channels are 3 anchors x 15
roles (iou, dx, dy, dw, dh, 10 class scores).  Each role is gathered into an
SBUF tile [P, F] where the partition dim enumerates (g, b, a) (g = half of the
cell space, to use 96 of 128 partitions) and the free dim enumerates cells.
Decode is then elementwise over role planes:

  pre   = grid*stride + stride*dx          (iota grid + ACT copy-scale + DVE add)
  half  = exp(dw + ln(anchor/2))           (single ACT op, bias folds the mul)
  x1/x2 = pre -/+ half
  mask  = iou > thresh
  kind  = tournament argmax over the 10 class planes

Outputs are written as dense planes ob[6, B, 3, HW] (+ mask [B, 3, HW]) per
head; the host interleaves them into the reference [N, 6] ordering (cheap
numpy transpose, keeps every DMA fully contiguous).
"""

import numpy as np

import concourse.bass as bass
import concourse.bacc as bacc
import concourse.mybir as mybir
from concourse.tile import TileContext
from concourse.bass_utils import run_bass_kernel_spmd

F32 = mybir.dt.float32
U8 = mybir.dt.uint8
ALU = mybir.AluOpType
ACTF = mybir.ActivationFunctionType

NCORES = 8
B = 128
BL = B // NCORES  # 16

# name, H(=W), stride, G (cell-space split to widen partitions), free chunk
HEADS = [
    ("13", 13, 32, 1, 169),
    ("26", 26, 16, 2, 338),
    ("52", 52, 8, 2, 676),
]


def _build_nc(reps=1):
    # Bacc (not raw Bass): its compile() legalizes multi-wait sync_info into
    # event semaphores (hardware allows one wait per instruction).
    # reps>1 wraps the body in a For_i loop -- used only for benchmarking
    # (fixed host/proxy overhead cancels between reps=1 and reps=R runs).
    import contextlib
    nc = bacc.Bacc("TRN2", target_bir_lowering=False)
    ins = {}
    obs = {}
    oms = {}
    for name, H, S, G, FC in HEADS:
        HW = H * H
        ins[name] = nc.declare_dram_parameter(f"in{name}", [BL, 45, HW], F32, isOutput=False)
        obs[name] = nc.declare_dram_parameter(f"ob{name}", [6, BL, 3, HW], F32, isOutput=True)
        oms[name] = nc.declare_dram_parameter(f"om{name}", [BL, 3, HW], U8, isOutput=True)
    consts = nc.declare_dram_parameter("consts", [96, 8], F32, isOutput=False)

    with TileContext(nc) as tc:
        with (
            tc.tile_pool(name="persist", bufs=1) as ppool,
            tc.tile_pool(name="work", bufs=2) as wpool,
            tc.tile_pool(name="scratch", bufs=1) as spool,
        ):
            consts_t = ppool.tile([96, 8], F32, tag="consts")
            nc.sync.dma_start(out=consts_t[:, :], in_=consts[:])

            # Per-head grid tiles (stride-scaled cell coordinates), built once
            # on gpsimd via iota.  Values <= 408, exact in f32.
            grids = {}
            for name, H, S, G, FC in HEADS:
                P = 48 * G
                H2 = H // G
                F = H2 * H
                gx = ppool.tile([P, F], F32, tag=f"gx{name}")
                gy = ppool.tile([P, F], F32, tag=f"gy{name}")
                nc.gpsimd.iota(
                    gx[:, :], pattern=[[0, H2], [S, H]], base=0,
                    channel_multiplier=0, allow_small_or_imprecise_dtypes=True,
                )
                # iota can't start at partition 48 (and partition windows must
                # be 32-aligned), so write the g=0 pattern everywhere; the
                # g=1 half-offset (H2*S = 208 for both split heads) is folded
                # into the sdy activation bias below via consts col 7.
                nc.gpsimd.iota(
                    gy[:, :], pattern=[[S, H2], [0, H]], base=0,
                    channel_multiplier=0, allow_small_or_imprecise_dtypes=True,
                )
                grids[name] = (gx, gy)

            rep_ctx = tc.For_i(0, reps, 1) if reps > 1 else contextlib.nullcontext()
            with rep_ctx:
                _emit_body(nc, tc, wpool, spool, consts_t, grids, ins, obs, oms)
    nc.compile()
    return nc


def _emit_body(nc, tc, wpool, spool, consts_t, grids, ins, obs, oms):
            for hi, (name, H, S, G, FC) in enumerate(HEADS):
                HW = H * H
                P = 48 * G
                F = HW // G
                gx, gy = grids[name]
                in_r = ins[name][:].rearrange("b (a r) (g f) -> g b a r f", a=3, g=G)
                ob_r = obs[name][:].rearrange("r b a (g f) -> r g b a f", g=G)
                om_r = oms[name][:].rearrange("b a (g f) -> g b a f", g=G)
                cw, ch = 2 * hi, 2 * hi + 1

                for cs in range(0, F, FC):
                    ce = cs + FC
                    t = []
                    for k in range(15):
                        tk = wpool.tile([P, FC], F32, tag=f"r{k}")
                        nc.sync.dma_start(out=tk[:, :], in_=in_r[:, :, :, k, cs:ce])
                        t.append(tk)

                    sdx = wpool.tile([P, FC], F32, tag="sdx")
                    sdy = wpool.tile([P, FC], F32, tag="sdy")
                    hw2 = wpool.tile([P, FC], F32, tag="hw2")
                    hh2 = wpool.tile([P, FC], F32, tag="hh2")
                    nc.scalar.activation(sdx[:, :], t[1][:, :], ACTF.Copy, scale=float(S))
                    # bias adds the g-half y offset (0 for g=0 / head13 rows)
                    nc.scalar.activation(sdy[:, :], t[2][:, :], ACTF.Identity,
                                         scale=float(S), bias=consts_t[0:P, 7:8])
                    nc.scalar.activation(hw2[:, :], t[3][:, :], ACTF.Exp, bias=consts_t[0:P, cw:cw + 1])
                    nc.scalar.activation(hh2[:, :], t[4][:, :], ACTF.Exp, bias=consts_t[0:P, ch:ch + 1])

                    # pre_x/pre_y in place on sdx/sdy
                    nc.vector.tensor_tensor(sdx[:, :], sdx[:, :], gx[:, cs:ce], ALU.add)
                    nc.vector.tensor_tensor(sdy[:, :], sdy[:, :], gy[:, cs:ce], ALU.add)
                    x1 = wpool.tile([P, FC], F32, tag="x1")
                    y1 = wpool.tile([P, FC], F32, tag="y1")
                    nc.vector.tensor_tensor(x1[:, :], sdx[:, :], hw2[:, :], ALU.subtract)
                    nc.vector.tensor_tensor(y1[:, :], sdy[:, :], hh2[:, :], ALU.subtract)
                    # x2/y2 in place on sdx/sdy
                    nc.vector.tensor_tensor(sdx[:, :], sdx[:, :], hw2[:, :], ALU.add)
                    nc.vector.tensor_tensor(sdy[:, :], sdy[:, :], hh2[:, :], ALU.add)

                    # mask = iou > thresh, without TensorScalarPtr (1 wait slot
                    # only): sign(iou - thresh) is 1 iff strictly greater,
                    # then relu maps {-1,0,1} -> {0,1}.  consts col 6 = -thresh.
                    msk = wpool.tile([P, FC], U8, tag="msk")
                    sgn = wpool.tile([P, FC], F32, tag="sgn")
                    nc.scalar.activation(sgn[:, :], t[0][:, :], ACTF.Sign, bias=consts_t[0:P, 6:7])
                    nc.scalar.activation(msk[:, :], sgn[:, :], ACTF.Relu)

                    # tournament argmax over class planes t[5..14]
                    c = t[5:]
                    m = []   # running max planes (in place on even class planes)
                    ix = []  # running argmax planes
                    for i in range(5):
                        gt = spool.tile([P, FC], U8, tag=f"gt{i}")
                        idx = spool.tile([P, FC], F32, tag=f"idx{i}")
                        nc.vector.tensor_tensor(gt[:, :], c[2 * i + 1][:, :], c[2 * i][:, :], ALU.is_gt)
                        nc.vector.tensor_tensor(c[2 * i][:, :], c[2 * i][:, :], c[2 * i + 1][:, :], ALU.max)
                        nc.vector.tensor_scalar(
                            out=idx[:, :], in0=gt[:, :],
                            scalar1=float(2 * i), scalar2=None, op0=ALU.add,
                        )
                        m.append(c[2 * i])
                        ix.append(idx)

                    def merge(i, j):
                        g2 = spool.tile([P, FC], U8, tag="gm")
                        nc.vector.tensor_tensor(g2[:, :], m[j][:, :], m[i][:, :], ALU.is_gt)
                        nc.vector.copy_predicated(ix[i][:, :], g2[:, :], ix[j][:, :])
                        nc.vector.tensor_tensor(m[i][:, :], m[i][:, :], m[j][:, :], ALU.max)

                    merge(0, 1)
                    merge(2, 3)
                    merge(0, 2)
                    merge(0, 4)

                    for ri, plane in ((0, t[0]), (1, x1), (2, y1), (3, sdx), (4, sdy), (5, ix[0])):
                        nc.sync.dma_start(out=ob_r[ri, :, :, :, cs:ce], in_=plane[:, :])
                    nc.sync.dma_start(out=om_r[:, :, :, cs:ce], in_=msk[:, :])


_NC_CACHE = {}


def _get_nc(reps=1):
    if reps not in _NC_CACHE:
        _NC_CACHE[reps] = _build_nc(reps)
    return _NC_CACHE[reps]


def _host_inputs(output13, output26, output52, anchors13, anchors26, anchors52, thresh):
    consts = np.zeros((96, 8), np.float32)
    amod = np.arange(96) % 3
    for hi, anc in enumerate((anchors13, anchors26, anchors52)):
        anc = np.asarray(anc, np.float32)
        consts[:, 2 * hi] = np.log(anc[:, 0] / 2.0)[amod]
        consts[:, 2 * hi + 1] = np.log(anc[:, 1] / 2.0)[amod]
    consts[:, 6] = -np.float32(thresh)
    consts[:, 7] = np.where(np.arange(96) >= 48, 208.0, 0.0)
    full = {"13": np.asarray(output13, np.float32),
            "26": np.asarray(output26, np.float32),
            "52": np.asarray(output52, np.float32)}
    in_maps = []
    for cid in range(NCORES):
        m = {"consts": consts}
        for name in full:
            sl = np.ascontiguousarray(full[name][cid * BL:(cid + 1) * BL])
            m[f"in{name}"] = sl.reshape(BL, 45, -1)
        in_maps.append(m)
    return in_maps


def _assemble(results):
    boxes_parts = []
    mask_parts = []
    for name, H, S, G, FC in HEADS:
        ob = np.concatenate([r[f"ob{name}"] for r in results], axis=1)  # [6,128,3,HW]
        om = np.concatenate([r[f"om{name}"] for r in results], axis=0)  # [128,3,HW]
        boxes_parts.append(np.ascontiguousarray(ob.transpose(1, 3, 2, 0)).reshape(-1, 6))
        mask_parts.append((om.transpose(0, 2, 1) != 0).reshape(-1))
    return np.concatenate(boxes_parts, 0), np.concatenate(mask_parts, 0)


def _run(trace=False, **inputs):
    nc = _get_nc()
    in_maps = _host_inputs(**inputs)
    res = run_bass_kernel_spmd(nc, in_maps, list(range(NCORES)), trace=trace)
    out = _assemble(res.results)
    return out, res


def kernel(**inputs):
    out, _ = _run(trace=False, **inputs)
    return out


def kernel_traced(**inputs):
    return _run(trace=True, **inputs)
